# revision 14
# baseline (speedup 1.0000x reference)
"""Multi-head causal attention (B=2, S=2048, D=1024, H=16, DK=64) on 8 trn2 cores.

Sharding: 2-way data parallel over batch x 4-way tensor parallel over heads.
Core c handles batch b = c // 4 and head group hg = c % 4 (4 heads = 256 dims).

Per-core kernel (all in transposed "feature on partitions" layouts):
  QT = (Wq_c/8)^T-style projection:  QT[d2, n] accumulated over D in PSUM
  KT likewise; V in natural [n, d] layout, augmented with a leading ones
  column so the attnV matmul also produces the softmax denominator.
  Scores are computed directly transposed: S^T[k, q] = K_tile @ Q  (contract
  over head dim), causal-masked, exp'd on ACT (no max subtraction needed:
  scores are O(5) so fp32 exp cannot overflow), then
  O^T[d, q] = sum_t V_t^T-augmented @ exp(S^T_t)  accumulated in PSUM,
  row 0 of which is the denominator.  O^T is normalized in-place and the
  output projection contracts over local head dims.  Host sums the 4 head
  group partials per batch and adds the bias.
"""

import numpy as np

B, S, D, DK = 2, 2048, 1024, 64
H = D // DK  # 16
NCORES = 8
BATCH_SHARDS = 2
HEAD_SHARDS = 4
HL = H // HEAD_SHARDS  # heads per core
DL = HL * DK  # local head dims per core

import os as _os

_F32R = _os.environ.get("MHA_F32R", "1") == "1"  # float32r matmuls (full-rate PE)


def build_nc(s_core=S, d_model=D, hl=HL, f32r=_F32R):
    from contextlib import ExitStack

    import concourse.bacc as bacc
    import concourse.bass as bass
    import concourse.mybir as mybir
    import concourse.tile as tile

    f32 = mybir.dt.float32
    Exp = mybir.ActivationFunctionType.Exp

    dl = hl * DK
    nhb = max(1, dl // 128)  # 128-wide blocks of local head dims
    KB = d_model // 128  # contraction tiles for projections
    NT = s_core // 128  # token tiles
    QSB = 512  # query superblock
    NQSB = s_core // QSB
    NBC = s_core // 256  # phase-1 token chunks

    def mm(ap):
        return ap.bitcast(mybir.dt.float32r) if f32r else ap

    nc = bacc.Bacc("TRN2", target_bir_lowering=False, debug=False)
    xq = nc.declare_dram_parameter("xq", [d_model, s_core], f32, isOutput=False)
    xk = nc.declare_dram_parameter("xk", [d_model, s_core], f32, isOutput=False)
    xv = nc.declare_dram_parameter("xv", [d_model, s_core], f32, isOutput=False)
    wq = nc.declare_dram_parameter("wq", [d_model, dl], f32, isOutput=False)
    wk = nc.declare_dram_parameter("wk", [d_model, dl], f32, isOutput=False)
    wv = nc.declare_dram_parameter("wv", [d_model, dl], f32, isOutput=False)
    wp = nc.declare_dram_parameter("wp", [dl, d_model], f32, isOutput=False)
    out = nc.declare_dram_parameter("out", [s_core, d_model], f32, isOutput=True)
    sums_dram = nc.dram_tensor("sums_dram", [hl, s_core], f32)

    with ExitStack() as ctx:
        tc = ctx.enter_context(tile.TileContext(nc))
        sb = ctx.enter_context(tc.tile_pool(name="sb", bufs=1))
        stream = ctx.enter_context(tc.tile_pool(name="stream", bufs=2))
        work = ctx.enter_context(tc.tile_pool(name="work", bufs=3))
        psum = ctx.enter_context(tc.tile_pool(name="psum", bufs=8, space="PSUM"))

        def bank():
            return psum.tile([128, 512], f32, tag="bank", bufs=8, name="bank")

        # ---- persistent SBUF state ----
        wq_sb = sb.tile([128, KB, dl], f32)
        wk_sb = sb.tile([128, KB, dl], f32)
        wv_sb = sb.tile([128, KB, dl], f32)
        nc.sync.dma_start(out=wq_sb, in_=wq[:, :].rearrange("(kb p) m -> p kb m", p=128))
        nc.sync.dma_start(out=wk_sb, in_=wk[:, :].rearrange("(kb p) m -> p kb m", p=128))
        nc.sync.dma_start(out=wv_sb, in_=wv[:, :].rearrange("(kb p) m -> p kb m", p=128))
        wp_sb = sb.tile([64, hl, d_model], f32)
        nc.sync.dma_start(out=wp_sb, in_=wp[:, :].rearrange("(h d) c -> d h c", d=64))

        qt_sb = sb.tile([128, nhb, s_core], f32)  # [d2, hb, n]
        kt_sb = sb.tile([128, nhb, s_core], f32)
        vaug = sb.tile([128, hl, NT, DK + 1], f32)  # [k, h, ktile, [d | 1]]
        nc.vector.memset(vaug[:, :, :, DK : DK + 1], 1.0)
        ot_sb = sb.tile([64, hl, s_core], f32)  # [d, h, n]

        # additive causal mask for the diagonal 128x128 blocks in [k, q] layout:
        # keep 0 where k <= q (p <= j), else -1e30
        mask_sb = sb.tile([128, 128], f32)
        nc.gpsimd.memset(mask_sb, 0.0)
        nc.gpsimd.affine_select(
            out=mask_sb,
            in_=mask_sb,
            compare_op=mybir.AluOpType.is_ge,
            fill=-1e30,
            base=0,
            pattern=[[1, 128]],
            channel_multiplier=-1,
        )

        # ---- phase 1: projections ----
        KH = KB // 2  # stream X in two half-contraction tiles per chunk
        for nb in range(NBC):
            n0 = nb * 256
            xts = {}
            for kh in range(2):
                r0 = kh * KH * 128
                xq_t = stream.tile([128, KH, 256], f32, tag=f"xq{kh}", name="xq_t")
                xk_t = stream.tile([128, KH, 256], f32, tag=f"xk{kh}", name="xk_t")
                xv_t = stream.tile([128, KH, 256], f32, tag=f"xv{kh}", name="xv_t")
                for t, src in ((xq_t, xq), (xk_t, xk), (xv_t, xv)):
                    nc.sync.dma_start(
                        out=t,
                        in_=src[r0 : r0 + KH * 128, n0 : n0 + 256].rearrange(
                            "(kb p) n -> p kb n", p=128
                        ),
                    )
                xts[kh] = (xq_t, xk_t, xv_t)
            ps_q = [bank() for _ in range(nhb)]
            ps_k = [bank() for _ in range(nhb)]
            for kb in range(KB):
                kh, kbl = divmod(kb, KH)
                xq_t, xk_t, xv_t = xts[kh]
                st, sp = kb == 0, kb == KB - 1
                for hb in range(nhb):
                    nc.tensor.matmul(
                        ps_q[hb][:, :256],
                        mm(wq_sb[:, kb, hb * 128 : hb * 128 + 128]),
                        mm(xq_t[:, kbl, :]),
                        start=st,
                        stop=sp,
                    )
                    nc.tensor.matmul(
                        ps_k[hb][:, :256],
                        mm(wk_sb[:, kb, hb * 128 : hb * 128 + 128]),
                        mm(xk_t[:, kbl, :]),
                        start=st,
                        stop=sp,
                    )
            for hb in range(nhb):
                nc.scalar.copy(out=qt_sb[:, hb, n0 : n0 + 256], in_=ps_q[hb][:, :256])
                nc.scalar.copy(out=kt_sb[:, hb, n0 : n0 + 256], in_=ps_k[hb][:, :256])
            for j in range(2):  # two 128-token tiles per chunk
                nt = nb * 2 + j
                ps_v = bank()
                for kb in range(KB):
                    kh, kbl = divmod(kb, KH)
                    nc.tensor.matmul(
                        ps_v[:, :dl],
                        mm(xts[kh][2][:, kbl, j * 128 : j * 128 + 128]),
                        mm(wv_sb[:, kb, :]),
                        start=kb == 0,
                        stop=kb == KB - 1,
                    )
                nc.vector.tensor_copy(
                    out=vaug[:, :, nt, 0:DK],
                    in_=ps_v[:, :dl].rearrange("p (h d) -> p h d", d=DK),
                )

        # ---- phase 2: causal attention per local head ----
        for h in range(hl):
            hb, ho = h // 2, (h % 2) * 64
            qt_h = qt_sb[ho : ho + 64, hb, :]
            kt_h = kt_sb[ho : ho + 64, hb, :]
            for qsb in range(NQSB):
                q0 = qsb * QSB
                po = bank()  # rows: [64 head dims | denominator]
                nkt = (qsb + 1) * (QSB // 128)
                for t in range(nkt):
                    r = t - qsb * (QSB // 128)
                    c0 = r * 128 if r > 0 else 0
                    ps = bank()
                    nc.tensor.matmul(
                        ps[:, c0:QSB],
                        mm(kt_h[:, t * 128 : t * 128 + 128]),
                        mm(qt_h[:, q0 + c0 : q0 + QSB]),
                        start=True,
                        stop=True,
                    )
                    if r >= 0:  # diagonal block: apply triangular causal mask
                        nc.vector.tensor_add(
                            out=ps[:, c0 : c0 + 128],
                            in0=ps[:, c0 : c0 + 128],
                            in1=mask_sb,
                        )
                    et = work.tile([128, QSB], f32, tag="et", bufs=3)
                    nc.scalar.activation(out=et[:, c0:QSB], in_=ps[:, c0:QSB], func=Exp)
                    nc.tensor.matmul(
                        po[0:65, c0:QSB],
                        mm(vaug[:, h, t, :]),
                        mm(et[:, c0:QSB]),
                        start=t == 0,
                        stop=t == nkt - 1,
                    )
                sums_q = work.tile([1, QSB], f32, tag="sums", bufs=2)
                nc.vector.tensor_copy(out=sums_q, in_=po[64:65, :])
                nc.sync.dma_start(out=sums_dram[h : h + 1, q0 : q0 + QSB], in_=sums_q)
                nc.vector.tensor_copy(
                    out=ot_sb[0:64, h, q0 : q0 + QSB], in_=po[0:64, :]
                )
            # normalize: ot[d, q] /= sums[q].  Broadcast the sums row across
            # 64 partitions via a DRAM bounce (partition-step-0 source AP),
            # take the reciprocal in place, and scale ot.
            rb = work.tile([64, s_core], f32, tag="rb", bufs=2)
            sd_row = sums_dram[h, :]
            sd_bcast = bass.AP(
                tensor=sd_row.tensor,
                offset=sd_row.offset,
                ap=[[0, 64]] + list(sd_row.ap),
            )
            nc.sync.dma_start(out=rb, in_=sd_bcast)
            nc.vector.reciprocal(out=rb, in_=rb)
            nc.vector.tensor_mul(
                out=ot_sb[0:64, h, :], in0=ot_sb[0:64, h, :], in1=rb
            )

        # ---- phase 3: output projection ----
        for nt in range(NT):
            os_t = work.tile([128, d_model], f32, tag="osb", bufs=2)
            for cb in range(d_model // 512):
                p3 = bank()
                for h in range(hl):
                    nc.tensor.matmul(
                        p3,
                        mm(ot_sb[0:64, h, nt * 128 : nt * 128 + 128]),
                        mm(wp_sb[0:64, h, cb * 512 : cb * 512 + 512]),
                        start=h == 0,
                        stop=h == hl - 1,
                    )
                nc.scalar.copy(out=os_t[:, cb * 512 : cb * 512 + 512], in_=p3)
            nc.sync.dma_start(out=out[nt * 128 : nt * 128 + 128, :], in_=os_t)

    nc.compile()
    return nc


_NC_CACHE = {}


def _get_nc():
    key = (S, D, HL, _F32R)
    if key not in _NC_CACHE:
        _NC_CACHE[key] = build_nc()
    return _NC_CACHE[key]


def shard_inputs(query_data, key_data, value_data, Wq, Wk, Wv, Wp):
    """Build the 8 per-core input maps."""
    qd = np.asarray(query_data, np.float32)
    kd = np.asarray(key_data, np.float32)
    vd = np.asarray(value_data, np.float32)
    Wqs = np.asarray(Wq, np.float32) * (1.0 / np.sqrt(DK))  # fold score scale into Wq
    Wk = np.asarray(Wk, np.float32)
    Wv = np.asarray(Wv, np.float32)
    Wp = np.asarray(Wp, np.float32)

    xqT = [np.ascontiguousarray(qd[b].T) for b in range(B)]
    xkT = [np.ascontiguousarray(kd[b].T) for b in range(B)]
    xvT = [np.ascontiguousarray(vd[b].T) for b in range(B)]

    in_maps = []
    for c in range(NCORES):
        b, hg = divmod(c, HEAD_SHARDS)
        cs = slice(hg * DL, (hg + 1) * DL)
        in_maps.append(
            {
                "xq": xqT[b],
                "xk": xkT[b],
                "xv": xvT[b],
                "wq": np.ascontiguousarray(Wqs[:, cs]),
                "wk": np.ascontiguousarray(Wk[:, cs]),
                "wv": np.ascontiguousarray(Wv[:, cs]),
                "wp": np.ascontiguousarray(Wp[cs, :]),
            }
        )
    return in_maps


def kernel(query_data, key_data, value_data, Wq, Wk, Wv, Wp, bp):
    from concourse.bass_utils import run_bass_kernel_spmd

    nc = _get_nc()
    in_maps = shard_inputs(query_data, key_data, value_data, Wq, Wk, Wv, Wp)
    res = run_bass_kernel_spmd(nc, in_maps, list(range(NCORES))).results
    out = np.zeros((B, S, D), np.float32)
    for c in range(NCORES):
        b = c // HEAD_SHARDS
        out[b] += res[c]["out"]
    out += np.asarray(bp, np.float32)
    return out


# revision 16
# speedup vs baseline: 1.0833x; 1.0833x over previous
"""Multi-head causal attention (B=2, S=2048, D=1024, H=16, DK=64) on 8 trn2 cores.

Sharding: 2-way data parallel over batch x 4-way tensor parallel over heads.
Core c handles batch b = c // 4 and head group hg = c % 4 (4 heads = 256 dims).

Per-core kernel (all in transposed "feature on partitions" layouts):
  QT = (Wq_c/8)^T-style projection:  QT[d2, n] accumulated over D in PSUM
  KT likewise; V in natural [n, d] layout, augmented with a leading ones
  column so the attnV matmul also produces the softmax denominator.
  Scores are computed directly transposed: S^T[k, q] = K_tile @ Q  (contract
  over head dim), causal-masked, exp'd on ACT (no max subtraction needed:
  scores are O(5) so fp32 exp cannot overflow), then
  O^T[d, q] = sum_t V_t^T-augmented @ exp(S^T_t)  accumulated in PSUM,
  row 0 of which is the denominator.  O^T is normalized in-place and the
  output projection contracts over local head dims.  Host sums the 4 head
  group partials per batch and adds the bias.
"""

import numpy as np

B, S, D, DK = 2, 2048, 1024, 64
H = D // DK  # 16
NCORES = 8
BATCH_SHARDS = 2
HEAD_SHARDS = 4
HL = H // HEAD_SHARDS  # heads per core
DL = HL * DK  # local head dims per core

import os as _os

_F32R = _os.environ.get("MHA_F32R", "1") == "1"  # float32r matmuls (full-rate PE)


def build_nc(s_core=S, d_model=D, hl=HL, f32r=_F32R):
    from contextlib import ExitStack

    import concourse.bacc as bacc
    import concourse.bass as bass
    import concourse.mybir as mybir
    import concourse.tile as tile

    f32 = mybir.dt.float32
    mdt = mybir.dt.float32r if f32r else f32  # dtype of matmul operands
    Exp = mybir.ActivationFunctionType.Exp

    dl = hl * DK
    nhb = max(1, dl // 128)  # 128-wide blocks of local head dims
    KB = d_model // 128  # contraction tiles for projections
    NT = s_core // 128  # token tiles
    QSB = 512  # query superblock
    NQSB = s_core // QSB
    NBC = s_core // 256  # phase-1 token chunks

    nc = bacc.Bacc("TRN2", target_bir_lowering=False, debug=False)
    xq = nc.declare_dram_parameter("xq", [d_model, s_core], mdt, isOutput=False)
    xk = nc.declare_dram_parameter("xk", [d_model, s_core], mdt, isOutput=False)
    xv = nc.declare_dram_parameter("xv", [d_model, s_core], mdt, isOutput=False)
    wq = nc.declare_dram_parameter("wq", [d_model, dl], mdt, isOutput=False)
    wk = nc.declare_dram_parameter("wk", [d_model, dl], mdt, isOutput=False)
    wv = nc.declare_dram_parameter("wv", [d_model, dl], mdt, isOutput=False)
    wp = nc.declare_dram_parameter("wp", [dl, d_model], mdt, isOutput=False)
    out = nc.declare_dram_parameter("out", [s_core, d_model], f32, isOutput=True)
    sums_dram = nc.dram_tensor("sums_dram", [hl, s_core], f32)

    with ExitStack() as ctx:
        tc = ctx.enter_context(tile.TileContext(nc))
        sb = ctx.enter_context(tc.tile_pool(name="sb", bufs=1))
        stream = ctx.enter_context(tc.tile_pool(name="stream", bufs=2))
        work = ctx.enter_context(tc.tile_pool(name="work", bufs=3))
        psum = ctx.enter_context(tc.tile_pool(name="psum", bufs=8, space="PSUM"))

        def bank():
            return psum.tile([128, 512], f32, tag="bank", bufs=8, name="bank")

        # ---- persistent SBUF state ----
        wq_sb = sb.tile([128, KB, dl], mdt)
        wk_sb = sb.tile([128, KB, dl], mdt)
        wv_sb = sb.tile([128, KB, dl], mdt)
        nc.sync.dma_start(out=wq_sb, in_=wq[:, :].rearrange("(kb p) m -> p kb m", p=128))
        nc.sync.dma_start(out=wk_sb, in_=wk[:, :].rearrange("(kb p) m -> p kb m", p=128))
        nc.sync.dma_start(out=wv_sb, in_=wv[:, :].rearrange("(kb p) m -> p kb m", p=128))
        wp_sb = sb.tile([64, hl, d_model], mdt)
        nc.sync.dma_start(out=wp_sb, in_=wp[:, :].rearrange("(h d) c -> d h c", d=64))

        qt_sb = sb.tile([128, nhb, s_core], mdt)  # [d2, hb, n]
        kt_sb = sb.tile([128, nhb, s_core], mdt)
        vaug = sb.tile([128, hl, NT, DK + 1], mdt)  # [k, h, ktile, [d | 1]]
        nc.vector.memset(vaug[:, :, :, DK : DK + 1].bitcast(f32), 1.0)
        ot_sb = sb.tile([64, hl, s_core], mdt)  # [d, h, n]

        # additive causal mask for the diagonal 128x128 blocks in [k, q] layout:
        # keep 0 where k <= q (p <= j), else -1e30
        mask_sb = sb.tile([128, 128], f32)
        nc.gpsimd.memset(mask_sb, 0.0)
        nc.gpsimd.affine_select(
            out=mask_sb,
            in_=mask_sb,
            compare_op=mybir.AluOpType.is_ge,
            fill=-1e30,
            base=0,
            pattern=[[1, 128]],
            channel_multiplier=-1,
        )

        # ---- phase 1: projections ----
        KH = KB // 2  # stream X in two half-contraction tiles per chunk
        for nb in range(NBC):
            n0 = nb * 256
            xts = {}
            for kh in range(2):
                r0 = kh * KH * 128
                xq_t = stream.tile([128, KH, 256], mdt, tag=f"xq{kh}", name="xq_t")
                xk_t = stream.tile([128, KH, 256], mdt, tag=f"xk{kh}", name="xk_t")
                xv_t = stream.tile([128, KH, 256], mdt, tag=f"xv{kh}", name="xv_t")
                for t, src in ((xq_t, xq), (xk_t, xk), (xv_t, xv)):
                    nc.sync.dma_start(
                        out=t,
                        in_=src[r0 : r0 + KH * 128, n0 : n0 + 256].rearrange(
                            "(kb p) n -> p kb n", p=128
                        ),
                    )
                xts[kh] = (xq_t, xk_t, xv_t)
            ps_q = [bank() for _ in range(nhb)]
            ps_k = [bank() for _ in range(nhb)]
            for kb in range(KB):
                kh, kbl = divmod(kb, KH)
                xq_t, xk_t, xv_t = xts[kh]
                st, sp = kb == 0, kb == KB - 1
                for hb in range(nhb):
                    nc.tensor.matmul(
                        ps_q[hb][:, :256],
                        (wq_sb[:, kb, hb * 128 : hb * 128 + 128]),
                        (xq_t[:, kbl, :]),
                        start=st,
                        stop=sp,
                    )
                    nc.tensor.matmul(
                        ps_k[hb][:, :256],
                        (wk_sb[:, kb, hb * 128 : hb * 128 + 128]),
                        (xk_t[:, kbl, :]),
                        start=st,
                        stop=sp,
                    )
            for hb in range(nhb):
                nc.scalar.copy(out=qt_sb[:, hb, n0 : n0 + 256], in_=ps_q[hb][:, :256])
                nc.scalar.copy(out=kt_sb[:, hb, n0 : n0 + 256], in_=ps_k[hb][:, :256])
            for j in range(2):  # two 128-token tiles per chunk
                nt = nb * 2 + j
                ps_v = bank()
                for kb in range(KB):
                    kh, kbl = divmod(kb, KH)
                    nc.tensor.matmul(
                        ps_v[:, :dl],
                        (xts[kh][2][:, kbl, j * 128 : j * 128 + 128]),
                        (wv_sb[:, kb, :]),
                        start=kb == 0,
                        stop=kb == KB - 1,
                    )
                nc.vector.tensor_copy(
                    out=vaug[:, :, nt, 0:DK],
                    in_=ps_v[:, :dl].rearrange("p (h d) -> p h d", d=DK),
                )

        # ---- phase 2: causal attention per local head ----
        for h in range(hl):
            hb, ho = h // 2, (h % 2) * 64
            qt_h = qt_sb[ho : ho + 64, hb, :]
            kt_h = kt_sb[ho : ho + 64, hb, :]
            for qsb in range(NQSB):
                q0 = qsb * QSB
                po = bank()  # rows: [64 head dims | denominator]
                nkt = (qsb + 1) * (QSB // 128)
                for t in range(nkt):
                    r = t - qsb * (QSB // 128)
                    c0 = r * 128 if r > 0 else 0
                    ps = bank()
                    nc.tensor.matmul(
                        ps[:, c0:QSB],
                        (kt_h[:, t * 128 : t * 128 + 128]),
                        (qt_h[:, q0 + c0 : q0 + QSB]),
                        start=True,
                        stop=True,
                    )
                    if r >= 0:  # diagonal block: apply triangular causal mask
                        nc.vector.tensor_add(
                            out=ps[:, c0 : c0 + 128],
                            in0=ps[:, c0 : c0 + 128],
                            in1=mask_sb,
                        )
                    et = work.tile([128, QSB], mdt, tag="et", bufs=3)
                    nc.scalar.activation(out=et[:, c0:QSB], in_=ps[:, c0:QSB], func=Exp)
                    nc.tensor.matmul(
                        po[0:65, c0:QSB],
                        (vaug[:, h, t, :]),
                        (et[:, c0:QSB]),
                        start=t == 0,
                        stop=t == nkt - 1,
                    )
                sums_q = work.tile([1, QSB], f32, tag="sums", bufs=2)
                nc.vector.tensor_copy(out=sums_q, in_=po[64:65, :])
                nc.sync.dma_start(out=sums_dram[h : h + 1, q0 : q0 + QSB], in_=sums_q)
                nc.vector.tensor_copy(
                    out=ot_sb[0:64, h, q0 : q0 + QSB], in_=po[0:64, :]
                )
            # normalize: ot[d, q] /= sums[q].  Broadcast the sums row across
            # 64 partitions via a DRAM bounce (partition-step-0 source AP),
            # take the reciprocal in place, and scale ot.
            rb = work.tile([64, s_core], f32, tag="rb", bufs=2)
            sd_row = sums_dram[h, :]
            sd_bcast = bass.AP(
                tensor=sd_row.tensor,
                offset=sd_row.offset,
                ap=[[0, 64]] + list(sd_row.ap),
            )
            nc.sync.dma_start(out=rb, in_=sd_bcast)
            nc.vector.reciprocal(out=rb, in_=rb)
            nc.vector.tensor_mul(
                out=ot_sb[0:64, h, :], in0=ot_sb[0:64, h, :], in1=rb
            )

        # ---- phase 3: output projection ----
        for nt in range(NT):
            os_t = work.tile([128, d_model], f32, tag="osb", bufs=2)
            for cb in range(d_model // 512):
                p3 = bank()
                for h in range(hl):
                    nc.tensor.matmul(
                        p3,
                        (ot_sb[0:64, h, nt * 128 : nt * 128 + 128]),
                        (wp_sb[0:64, h, cb * 512 : cb * 512 + 512]),
                        start=h == 0,
                        stop=h == hl - 1,
                    )
                nc.scalar.copy(out=os_t[:, cb * 512 : cb * 512 + 512], in_=p3)
            nc.sync.dma_start(out=out[nt * 128 : nt * 128 + 128, :], in_=os_t)

    nc.compile()
    return nc


_NC_CACHE = {}


def _get_nc():
    key = (S, D, HL, _F32R)
    if key not in _NC_CACHE:
        _NC_CACHE[key] = build_nc()
    return _NC_CACHE[key]


def shard_inputs(query_data, key_data, value_data, Wq, Wk, Wv, Wp):
    """Build the 8 per-core input maps."""
    qd = np.asarray(query_data, np.float32)
    kd = np.asarray(key_data, np.float32)
    vd = np.asarray(value_data, np.float32)
    Wqs = np.asarray(Wq, np.float32) * (1.0 / np.sqrt(DK))  # fold score scale into Wq
    Wk = np.asarray(Wk, np.float32)
    Wv = np.asarray(Wv, np.float32)
    Wp = np.asarray(Wp, np.float32)

    xqT = [np.ascontiguousarray(qd[b].T) for b in range(B)]
    xkT = [np.ascontiguousarray(kd[b].T) for b in range(B)]
    xvT = [np.ascontiguousarray(vd[b].T) for b in range(B)]

    in_maps = []
    for c in range(NCORES):
        b, hg = divmod(c, HEAD_SHARDS)
        cs = slice(hg * DL, (hg + 1) * DL)
        in_maps.append(
            {
                "xq": xqT[b],
                "xk": xkT[b],
                "xv": xvT[b],
                "wq": np.ascontiguousarray(Wqs[:, cs]),
                "wk": np.ascontiguousarray(Wk[:, cs]),
                "wv": np.ascontiguousarray(Wv[:, cs]),
                "wp": np.ascontiguousarray(Wp[cs, :]),
            }
        )
    return in_maps


def kernel(query_data, key_data, value_data, Wq, Wk, Wv, Wp, bp):
    from concourse.bass_utils import run_bass_kernel_spmd

    nc = _get_nc()
    in_maps = shard_inputs(query_data, key_data, value_data, Wq, Wk, Wv, Wp)
    res = run_bass_kernel_spmd(nc, in_maps, list(range(NCORES))).results
    out = np.zeros((B, S, D), np.float32)
    for c in range(NCORES):
        b = c // HEAD_SHARDS
        out[b] += res[c]["out"]
    out += np.asarray(bp, np.float32)
    return out
